# revision 22
# baseline (speedup 1.0000x reference)
"""Trainium2 Bass kernel for ParallelRetention.

Reference computation (per batch*head):
    msum   = rowsum(omask)                               # [S]
    decay  = omask / sqrt(msum)        (nan -> 0)
    ret    = (q @ k^T) * decay                           # [S, S]
    scale  = clip(|rowsum(ret)|, 1, inf)
    out    = (ret / scale) @ v                           # [S, D]

Algebraic restructure used here (raw = (q@k^T)*omask):
    out = (raw @ v) / max(|rowsum(raw)|, sqrt(msum))
so the S x S matrix is never normalized element-wise; only per-row stats are
needed.

Sharding: 16 heads over 8 cores -> 2 heads/core (SPMD, identical program).

Per (head, q-tile of 128 rows):
    PE  : qk = qT.T @ kT            [128, S] fp32 psum   (K = D = 64, fp16 in)
    DVE : raw16 = qk * omask        (scalar_tensor_tensor, accum -> rowraw)
    ACT : msum = rowsum(omask)      (activation Copy with accum_out)
    PE  : transpose raw16 in 128x128 fp16 blocks -> psum
    ACT/DVE: copy rawT psum -> sbuf (one half each)
    PE  : out_psum += rawT_chunk.T @ v16_chunk   (K = 128 per chunk)
    ACT : copy out_psum -> staging
Per head epilogue: factor = 1/max(|rowraw|, sqrt(msum)) batched on [128, nq],
one broadcast multiply, one output DMA.
"""

import os
from contextlib import ExitStack

import numpy as np

import concourse.bass as bass
import concourse.bacc as bacc
import concourse.tile as tile
from concourse import mybir
from concourse.bass_utils import run_bass_kernel_spmd
from concourse.masks import make_identity

F32 = mybir.dt.float32
F16 = mybir.dt.float16

B, H, S, D = 1, 16, 2048, 64
NCORES = 8
HPC = H // NCORES  # heads per core
P = 128


def build_retention(nc: bass.Bass, hpc: int = HPC, s: int = S, d: int = D,
                    reps: int = 1, msum_mode: str = "pool"):
    """Emit the retention kernel IR into `nc`.

    reps > 1 wraps the whole computation in a hardware loop — used only for
    timing (device time scales with reps while host/transfer overhead does
    not)."""
    nq = s // P   # query tiles per head
    nt = s // P   # t (key) chunks per head
    half = s // 2
    ob = min(4, nq)  # omask q-tiles per DMA chunk

    q_d = nc.declare_dram_parameter("q", [hpc, s, d], F32, isOutput=False)
    k_d = nc.declare_dram_parameter("k", [hpc, s, d], F32, isOutput=False)
    v_d = nc.declare_dram_parameter("v", [hpc, s, d], F32, isOutput=False)
    om_d = nc.declare_dram_parameter("omask", [hpc, s, s], F32, isOutput=False)
    out_d = nc.declare_dram_parameter("out", [hpc, s, d], F32, isOutput=True)

    with tile.TileContext(nc) as tc, ExitStack() as ctx:
        singles = ctx.enter_context(tc.tile_pool(name="singles", bufs=1))
        kq_pool = ctx.enter_context(tc.tile_pool(name="kq", bufs=2))
        ld_pool = ctx.enter_context(tc.tile_pool(name="ld", bufs=2))
        om_pool = ctx.enter_context(tc.tile_pool(name="om", bufs=2))
        raw_pool = ctx.enter_context(tc.tile_pool(name="raw", bufs=3))
        rawt_pool = ctx.enter_context(tc.tile_pool(name="rawt", bufs=3))
        stat_pool = ctx.enter_context(tc.tile_pool(name="stat", bufs=2))
        ostg_pool = ctx.enter_context(tc.tile_pool(name="ostg", bufs=2))
        scr_pool = ctx.enter_context(tc.tile_pool(name="scr", bufs=3))
        psum_qk = ctx.enter_context(tc.tile_pool(name="psqk", bufs=2, space="PSUM"))
        psum_rt = ctx.enter_context(tc.tile_pool(name="psrt", bufs=2, space="PSUM"))
        psum_out = ctx.enter_context(tc.tile_pool(name="psout", bufs=2, space="PSUM"))

        ident32 = singles.tile([P, P], F32)
        make_identity(nc, ident32)
        ident16 = singles.tile([P, P], F16)
        nc.scalar.copy(out=ident16, in_=ident32)

        if reps > 1:
            ctx.enter_context(tc.For_i(0, reps, 1))

        for h in range(hpc):
            # ---------- head setup: kT, qT [d, s] fp16; v16 [P, nt, d] fp16
            # One DMA per tensor; PE transposes 128x64 fp32 tiles (4 per psum
            # bank); the psum->sbuf copy casts to fp16.
            kT = kq_pool.tile([d, s], F16, tag="kT")
            qT = kq_pool.tile([d, s], F16, tag="qT")
            grp = min(4, nq)
            for src, dstT, tg in ((k_d, kT, "ksb"), (q_d, qT, "qsb")):
                sb = ld_pool.tile([P, nq, d], F32, tag=tg)
                nc.scalar.dma_start(out=sb, in_=src[h].rearrange("(i p) d -> p i d", p=P))
                for i4 in range(nq // grp):
                    ps4 = psum_rt.tile([P, grp * P], F32, tag="rt")
                    for j in range(grp):
                        i = i4 * grp + j
                        nc.tensor.transpose(ps4[:d, j * P:(j + 1) * P],
                                            sb[:, i, :], ident32)
                    nc.scalar.copy(out=dstT[:, i4 * grp * P:(i4 + 1) * grp * P],
                                   in_=ps4[:d, :])

            v32 = ld_pool.tile([P, nt, d], F32, tag="v32")
            nc.scalar.dma_start(out=v32, in_=v_d[h].rearrange("(t p) d -> p t d", p=P))
            v16 = kq_pool.tile([P, nt, d], F16, tag="v16")
            nc.scalar.copy(out=v16, in_=v32)

            # per-head stat tiles (filled per q-tile, consumed in epilogue)
            msum_all = stat_pool.tile([P, nq], F32, tag="msum")
            rra = stat_pool.tile([P, nq], F32, tag="rra")
            rrb = stat_pool.tile([P, nq], F32, tag="rrb")
            ostage = ostg_pool.tile([P, nq, d], F32, tag="ostage")

            # ---------- main loop over q tiles
            for i in range(nq):
                ib, off = divmod(i, ob)
                if off == 0:
                    om4 = om_pool.tile([P, ob, s], F32, tag="om")
                    dma_eng = nc.sync if ib % 2 == 0 else nc.scalar
                    dma_eng.dma_start(
                        out=om4,
                        in_=om_d[h, ib * ob * P:(ib + 1) * ob * P, :]
                        .rearrange("(b p) t -> p b t", p=P))
                om = om4[:, off, :]

                # qk = q_tile @ k^T  -> psum, two halves (2 banks each)
                lhs_q = qT[:, i * P:(i + 1) * P]
                qk_a = psum_qk.tile([P, half], F32, tag="qk")
                qk_b = psum_qk.tile([P, half], F32, tag="qk")
                nstep = min(512, half)
                for jhalf, qk in ((0, qk_a), (1, qk_b)):
                    for jj in range(half // nstep):
                        col = jhalf * half + jj * nstep
                        nc.tensor.matmul(
                            qk[:, jj * nstep:(jj + 1) * nstep],
                            lhs_q, kT[:, col:col + nstep],
                            start=True, stop=True,
                        )

                # msum = rowsum(omask)
                if msum_mode == "pool":
                    # fold 2:1 on the otherwise-idle gpsimd engine, finish the
                    # rowsum on ACT over half the data
                    fold = scr_pool.tile([P, half], F32, tag="fold")
                    nc.gpsimd.tensor_tensor(out=fold, in0=om[:, 0:half],
                                            in1=om[:, half:s],
                                            op=mybir.AluOpType.add)
                    om_scr = scr_pool.tile([P, half], F16, tag="omscr")
                    nc.scalar.activation(
                        om_scr, fold, mybir.ActivationFunctionType.Copy,
                        accum_out=msum_all[:, i:i + 1],
                    )
                else:
                    om_scr = scr_pool.tile([P, s], F16, tag="omscr")
                    nc.scalar.activation(
                        om_scr, om, mybir.ActivationFunctionType.Copy,
                        accum_out=msum_all[:, i:i + 1],
                    )

                # raw16 = qk * omask (fp16), accums -> rowsum halves
                raw16 = raw_pool.tile([P, s], F16, tag="raw")
                nc.vector.scalar_tensor_tensor(
                    out=raw16[:, 0:half], in0=qk_a, scalar=1.0,
                    in1=om[:, 0:half],
                    op0=mybir.AluOpType.mult, op1=mybir.AluOpType.mult,
                    accum_out=rra[:, i:i + 1],
                )
                nc.vector.scalar_tensor_tensor(
                    out=raw16[:, half:s], in0=qk_b, scalar=1.0,
                    in1=om[:, half:s],
                    op0=mybir.AluOpType.mult, op1=mybir.AluOpType.mult,
                    accum_out=rrb[:, i:i + 1],
                )

                # transpose raw16 into [t, sq] chunks; copy psum -> sbuf
                # (one half on ACT, one on DVE)
                rawT = rawt_pool.tile([P, s], F16, tag="rawT")
                for hh in range(2):
                    pst = psum_rt.tile([P, half], F16, tag="rt")
                    for tb in range(half // P):
                        sl = slice(tb * P, (tb + 1) * P)
                        src_sl = slice(hh * half + tb * P, hh * half + (tb + 1) * P)
                        nc.tensor.transpose(pst[:, sl], raw16[:, src_sl], ident16)
                    dst = rawT[:, hh * half:(hh + 1) * half]
                    if hh == 0:
                        nc.scalar.copy(out=dst, in_=pst)
                    else:
                        nc.vector.tensor_copy(out=dst, in_=pst)

                # out_psum += rawT_chunk.T @ v16_chunk
                ops = psum_out.tile([P, d], F32, tag="out")
                for t in range(nt):
                    nc.tensor.matmul(
                        ops, rawT[:, t * P:(t + 1) * P], v16[:, t, :],
                        start=(t == 0), stop=(t == nt - 1),
                    )
                nc.scalar.copy(out=ostage[:, i, :], in_=ops)

            # ---------- head epilogue: factor + scale + one output DMA
            # factor = 1 / max(|rowraw|, sqrt(msum)), batched on [P, nq]
            rowraw = stat_pool.tile([P, nq], F32, tag="rowraw")
            nc.vector.tensor_tensor(out=rowraw, in0=rra, in1=rrb,
                                    op=mybir.AluOpType.add)
            smsum = stat_pool.tile([P, nq], F32, tag="smsum")
            nc.scalar.activation(smsum, msum_all,
                                 mybir.ActivationFunctionType.Sqrt)
            arr = stat_pool.tile([P, nq], F32, tag="arr")
            nc.vector.scalar_tensor_tensor(
                out=arr, in0=rowraw, scalar=-1.0, in1=rowraw,
                op0=mybir.AluOpType.mult, op1=mybir.AluOpType.max)
            mx = stat_pool.tile([P, nq], F32, tag="mx")
            nc.vector.tensor_tensor(out=mx, in0=arr, in1=smsum,
                                    op=mybir.AluOpType.max)
            fac = stat_pool.tile([P, nq], F32, tag="fac")
            nc.vector.reciprocal(fac, mx)

            out_f = ostg_pool.tile([P, nq, d], F32, tag="outf")
            nc.vector.tensor_tensor(out=out_f, in0=ostage,
                                    in1=fac.to_broadcast([P, nq, d]),
                                    op=mybir.AluOpType.mult)
            nc.scalar.dma_start(
                out=out_d[h].rearrange("(i p) d -> p i d", p=P), in_=out_f)

    return nc


_CACHE = {}


def _get_nc():
    key = "main"
    if key not in _CACHE:
        nc = bacc.Bacc()
        build_retention(nc)
        nc.compile()
        _CACHE[key] = nc
    return _CACHE[key]


def kernel(q, k, v, omask):
    nc = _get_nc()
    in_maps = []
    for c in range(NCORES):
        hs = slice(c * HPC, (c + 1) * HPC)
        in_maps.append({
            "q": np.ascontiguousarray(q[0, hs]),
            "k": np.ascontiguousarray(k[0, hs]),
            "v": np.ascontiguousarray(v[0, hs]),
            "omask": np.ascontiguousarray(omask[0, hs]),
        })
    res = run_bass_kernel_spmd(nc, in_maps, list(range(NCORES)))
    outs = [res.results[c]["out"] for c in range(NCORES)]
    return np.concatenate(outs, axis=0).reshape(B, H, S, D)


# revision 23
# speedup vs baseline: 1.4446x; 1.4446x over previous
"""Trainium2 Bass kernel for ParallelRetention.

Reference computation (per batch*head):
    msum   = rowsum(omask)                               # [S]
    decay  = omask / sqrt(msum)        (nan -> 0)
    ret    = (q @ k^T) * decay                           # [S, S]
    scale  = clip(|rowsum(ret)|, 1, inf)
    out    = (ret / scale) @ v                           # [S, D]

Algebraic restructure used here (raw = (q@k^T)*omask):
    out = (raw @ v) / max(|rowsum(raw)|, sqrt(msum))
so the S x S matrix is never normalized element-wise; only per-row stats are
needed.

Sharding: 16 heads over 8 cores -> 2 heads/core (SPMD, identical program).

Per (head, q-tile of 128 rows):
    PE  : qk = qT.T @ kT            [128, S] fp32 psum   (K = D = 64, fp16 in)
    DVE : raw16 = qk * omask        (scalar_tensor_tensor, accum -> rowraw)
    ACT : msum = rowsum(omask)      (activation Copy with accum_out)
    PE  : transpose raw16 in 128x128 fp16 blocks -> psum
    ACT/DVE: copy rawT psum -> sbuf (one half each)
    PE  : out_psum += rawT_chunk.T @ v16_chunk   (K = 128 per chunk)
    ACT : copy out_psum -> staging
Per head epilogue: factor = 1/max(|rowraw|, sqrt(msum)) batched on [128, nq],
one broadcast multiply, one output DMA.
"""

import os
from contextlib import ExitStack

import numpy as np

import concourse.bass as bass
import concourse.bacc as bacc
import concourse.tile as tile
from concourse import mybir
from concourse.bass_utils import run_bass_kernel_spmd
from concourse.masks import make_identity

F32 = mybir.dt.float32
F16 = mybir.dt.float16

B, H, S, D = 1, 16, 2048, 64
NCORES = 8
HPC = H // NCORES  # heads per core
P = 128


def build_retention(nc: bass.Bass, hpc: int = HPC, s: int = S, d: int = D,
                    reps: int = 1, msum_mode: str = "act"):
    """Emit the retention kernel IR into `nc`.

    reps > 1 wraps the whole computation in a hardware loop — used only for
    timing (device time scales with reps while host/transfer overhead does
    not)."""
    nq = s // P   # query tiles per head
    nt = s // P   # t (key) chunks per head
    half = s // 2
    ob = min(4, nq)  # omask q-tiles per DMA chunk

    q_d = nc.declare_dram_parameter("q", [hpc, s, d], F32, isOutput=False)
    k_d = nc.declare_dram_parameter("k", [hpc, s, d], F32, isOutput=False)
    v_d = nc.declare_dram_parameter("v", [hpc, s, d], F32, isOutput=False)
    om_d = nc.declare_dram_parameter("omask", [hpc, s, s], F32, isOutput=False)
    out_d = nc.declare_dram_parameter("out", [hpc, s, d], F32, isOutput=True)

    with tile.TileContext(nc) as tc, ExitStack() as ctx:
        singles = ctx.enter_context(tc.tile_pool(name="singles", bufs=1))
        kq_pool = ctx.enter_context(tc.tile_pool(name="kq", bufs=2))
        ld_pool = ctx.enter_context(tc.tile_pool(name="ld", bufs=2))
        om_pool = ctx.enter_context(tc.tile_pool(name="om", bufs=2))
        raw_pool = ctx.enter_context(tc.tile_pool(name="raw", bufs=2))
        rawt_pool = ctx.enter_context(tc.tile_pool(name="rawt", bufs=2))
        stat_pool = ctx.enter_context(tc.tile_pool(name="stat", bufs=2))
        ostg_pool = ctx.enter_context(tc.tile_pool(name="ostg", bufs=2))
        scr_pool = ctx.enter_context(tc.tile_pool(name="scr", bufs=2))
        psum_qk = ctx.enter_context(tc.tile_pool(name="psqk", bufs=2, space="PSUM"))
        psum_rt = ctx.enter_context(tc.tile_pool(name="psrt", bufs=2, space="PSUM"))
        psum_out = ctx.enter_context(tc.tile_pool(name="psout", bufs=2, space="PSUM"))

        ident32 = singles.tile([P, P], F32)
        make_identity(nc, ident32)
        ident16 = singles.tile([P, P], F16)
        nc.scalar.copy(out=ident16, in_=ident32)

        if reps > 1:
            ctx.enter_context(tc.For_i(0, reps, 1))

        for h in range(hpc):
            # ---------- head setup: kT, qT [d, s] fp16; v16 [P, nt, d] fp16
            # One DMA per tensor; PE transposes 128x64 fp32 tiles (4 per psum
            # bank); the psum->sbuf copy casts to fp16.
            kT = kq_pool.tile([d, s], F16, tag="kT")
            qT = kq_pool.tile([d, s], F16, tag="qT")
            grp = min(4, nq)
            for src, dstT, tg in ((k_d, kT, "ksb"), (q_d, qT, "qsb")):
                sb = ld_pool.tile([P, nq, d], F32, tag=tg)
                nc.sync.dma_start(out=sb, in_=src[h].rearrange("(i p) d -> p i d", p=P))
                for i4 in range(nq // grp):
                    ps4 = psum_rt.tile([P, grp * P], F32, tag="rt")
                    for j in range(grp):
                        i = i4 * grp + j
                        nc.tensor.transpose(ps4[:d, j * P:(j + 1) * P],
                                            sb[:, i, :], ident32)
                    nc.scalar.copy(out=dstT[:, i4 * grp * P:(i4 + 1) * grp * P],
                                   in_=ps4[:d, :])

            v32 = ld_pool.tile([P, nt, d], F32, tag="v32")
            nc.sync.dma_start(out=v32, in_=v_d[h].rearrange("(t p) d -> p t d", p=P))
            v16 = kq_pool.tile([P, nt, d], F16, tag="v16")
            nc.scalar.copy(out=v16, in_=v32)

            # per-head stat tiles (filled per q-tile, consumed in epilogue)
            msum_all = stat_pool.tile([P, nq], F32, tag="msum")
            rra = stat_pool.tile([P, nq], F32, tag="rra")
            rrb = stat_pool.tile([P, nq], F32, tag="rrb")
            ostage = ostg_pool.tile([P, nq, d], F32, tag="ostage")

            # ---------- main loop over q tiles
            for i in range(nq):
                ib, off = divmod(i, ob)
                if off == 0:
                    om4 = om_pool.tile([P, ob, s], F32, tag="om")
                    nc.sync.dma_start(
                        out=om4,
                        in_=om_d[h, ib * ob * P:(ib + 1) * ob * P, :]
                        .rearrange("(b p) t -> p b t", p=P))
                om = om4[:, off, :]

                # qk = q_tile @ k^T  -> psum, two halves (2 banks each)
                lhs_q = qT[:, i * P:(i + 1) * P]
                qk_a = psum_qk.tile([P, half], F32, tag="qk")
                qk_b = psum_qk.tile([P, half], F32, tag="qk")
                nstep = min(512, half)
                for jhalf, qk in ((0, qk_a), (1, qk_b)):
                    for jj in range(half // nstep):
                        col = jhalf * half + jj * nstep
                        nc.tensor.matmul(
                            qk[:, jj * nstep:(jj + 1) * nstep],
                            lhs_q, kT[:, col:col + nstep],
                            start=True, stop=True,
                        )

                # msum = rowsum(omask)
                if msum_mode == "pool":
                    # fold 2:1 on the otherwise-idle gpsimd engine, finish the
                    # rowsum on ACT over half the data
                    fold = scr_pool.tile([P, half], F32, tag="fold")
                    nc.gpsimd.tensor_tensor(out=fold, in0=om[:, 0:half],
                                            in1=om[:, half:s],
                                            op=mybir.AluOpType.add)
                    om_scr = scr_pool.tile([P, half], F16, tag="omscr")
                    nc.scalar.activation(
                        om_scr, fold, mybir.ActivationFunctionType.Copy,
                        accum_out=msum_all[:, i:i + 1],
                    )
                else:
                    om_scr = scr_pool.tile([P, s], F16, tag="omscr")
                    nc.scalar.activation(
                        om_scr, om, mybir.ActivationFunctionType.Copy,
                        accum_out=msum_all[:, i:i + 1],
                    )

                # raw16 = qk * omask (fp16), accums -> rowsum halves
                raw16 = raw_pool.tile([P, s], F16, tag="raw")
                nc.vector.scalar_tensor_tensor(
                    out=raw16[:, 0:half], in0=qk_a, scalar=1.0,
                    in1=om[:, 0:half],
                    op0=mybir.AluOpType.mult, op1=mybir.AluOpType.mult,
                    accum_out=rra[:, i:i + 1],
                )
                nc.vector.scalar_tensor_tensor(
                    out=raw16[:, half:s], in0=qk_b, scalar=1.0,
                    in1=om[:, half:s],
                    op0=mybir.AluOpType.mult, op1=mybir.AluOpType.mult,
                    accum_out=rrb[:, i:i + 1],
                )

                # transpose raw16 into [t, sq] chunks; copy psum -> sbuf
                # (one half on ACT, one on DVE)
                rawT = rawt_pool.tile([P, s], F16, tag="rawT")
                for hh in range(2):
                    pst = psum_rt.tile([P, half], F16, tag="rt")
                    for tb in range(half // P):
                        sl = slice(tb * P, (tb + 1) * P)
                        src_sl = slice(hh * half + tb * P, hh * half + (tb + 1) * P)
                        nc.tensor.transpose(pst[:, sl], raw16[:, src_sl], ident16)
                    dst = rawT[:, hh * half:(hh + 1) * half]
                    if hh == 0:
                        nc.scalar.copy(out=dst, in_=pst)
                    else:
                        nc.vector.tensor_copy(out=dst, in_=pst)

                # out_psum += rawT_chunk.T @ v16_chunk
                ops = psum_out.tile([P, d], F32, tag="out")
                for t in range(nt):
                    nc.tensor.matmul(
                        ops, rawT[:, t * P:(t + 1) * P], v16[:, t, :],
                        start=(t == 0), stop=(t == nt - 1),
                    )
                nc.scalar.copy(out=ostage[:, i, :], in_=ops)

            # ---------- head epilogue: factor + scale + one output DMA
            # factor = 1 / max(|rowraw|, sqrt(msum)), batched on [P, nq]
            rowraw = stat_pool.tile([P, nq], F32, tag="rowraw")
            nc.vector.tensor_tensor(out=rowraw, in0=rra, in1=rrb,
                                    op=mybir.AluOpType.add)
            smsum = stat_pool.tile([P, nq], F32, tag="smsum")
            nc.scalar.activation(smsum, msum_all,
                                 mybir.ActivationFunctionType.Sqrt)
            arr = stat_pool.tile([P, nq], F32, tag="arr")
            nc.vector.scalar_tensor_tensor(
                out=arr, in0=rowraw, scalar=-1.0, in1=rowraw,
                op0=mybir.AluOpType.mult, op1=mybir.AluOpType.max)
            mx = stat_pool.tile([P, nq], F32, tag="mx")
            nc.vector.tensor_tensor(out=mx, in0=arr, in1=smsum,
                                    op=mybir.AluOpType.max)
            fac = stat_pool.tile([P, nq], F32, tag="fac")
            nc.vector.reciprocal(fac, mx)

            out_f = ostg_pool.tile([P, nq, d], F32, tag="outf")
            nc.vector.tensor_tensor(out=out_f, in0=ostage,
                                    in1=fac.to_broadcast([P, nq, d]),
                                    op=mybir.AluOpType.mult)
            nc.sync.dma_start(
                out=out_d[h].rearrange("(i p) d -> p i d", p=P), in_=out_f)

    return nc


_CACHE = {}


def _get_nc():
    key = "main"
    if key not in _CACHE:
        nc = bacc.Bacc()
        build_retention(nc)
        nc.compile()
        _CACHE[key] = nc
    return _CACHE[key]


def kernel(q, k, v, omask):
    nc = _get_nc()
    in_maps = []
    for c in range(NCORES):
        hs = slice(c * HPC, (c + 1) * HPC)
        in_maps.append({
            "q": np.ascontiguousarray(q[0, hs]),
            "k": np.ascontiguousarray(k[0, hs]),
            "v": np.ascontiguousarray(v[0, hs]),
            "omask": np.ascontiguousarray(omask[0, hs]),
        })
    res = run_bass_kernel_spmd(nc, in_maps, list(range(NCORES)))
    outs = [res.results[c]["out"] for c in range(NCORES)]
    return np.concatenate(outs, axis=0).reshape(B, H, S, D)
